# revision 31
# baseline (speedup 1.0000x reference)
"""Piecewise-linear activation (uniform 16-point grid) on 8 trn2 NeuronCores.

Math: the reference is (except at exact grid hits) the continuous PWL function
    f(x) = A*x + B + sum_k c_k * relu(x - xs_k),   k = 0..15
with
    m_j   = (ys[j+1]-ys[j])/(xs[j+1]-xs[j])      (15 interior slopes)
    c_0   = m_0 - slopes[0]
    c_k   = m_k - m_{k-1}                         (k=1..14)
    c_15  = slopes[1] - m_14
    A     = slopes[0],  B = ys[0] - slopes[0]*xs[0]
At an exact interior grid hit x == xs[j] (j=1..15) the reference's
argmin/argmax tie-breaking yields the two-segment-wide interpolation value
(discontinuous from f); those are patched with EQ_SELECT fixup slots.

Engine split per 128x2048 supertile (driven by measured per-instr costs; the
PE array is HAM power-throttled and fp32 matmul is 2-pass, ~4.5us/st/term;
DVE RELU_MAC fuses produce+scale+accumulate into one ~2.4us 1x pass):
  ACT : one seed term c_k*relu(x-xs_k) (positive c_k, scale rides the ACT)
        + unscaled relu produces for the PE terms
        (+ A*x seed via Identity when A != 0)
  DVE : RELU_MAC chain for 9 terms on top of the seed, PSUM merge via
        affine_then_add whose bias slot carries B, then EQ_SELECT fixups
  PE  : 6 terms accumulate into PSUM via c_k*I fp32 matmuls
  (GPSIMD tensor ops are rejected by walrus codegen / crash the NC;
   accum-DMA works but serializes the output tile - both unused)
"""

import numpy as np
from contextlib import ExitStack

import concourse.bass as bass
import concourse.bacc as bacc
import concourse.tile as tile
from concourse import mybir
from concourse.bass_utils import run_bass_kernel_spmd

F32 = mybir.dt.float32
AF = mybir.ActivationFunctionType
OP = mybir.AluOpType

N_CORES = 8
P = 128
FREE = 8192              # elements per partition per core (1024*1024/128)
ST = 2048                # supertile free size
CHUNK = 512              # one PSUM bank worth of fp32

N_MAC = 7                # terms fused on the DVE RELU_MAC chain
N_PE = 6                 # PE matmul groups (owner terms)
N_PAIR = 2               # partner terms DMA-accumulated into the last owners
N_SLOTS = 2              # exact-hit fixup slots
UNUSED_G = 1.0e30        # sentinel no input value ever equals

# term order: [seed (positive c)] + N_MAC chain terms + N_PE terms
# consts column layout (each column replicated over 128 partitions):
NCOL = 48
COL_A = 0
COL_B = 1
COL_BIAS = 2             # 16 cols: -xs_k in term order
COL_C = 18               # 16 cols: c_k in term order
COL_SEEDB = 34           # -c_seed * xs_seed (ACT bias for the scaled seed)
COL_PSC = 35             # N_PAIR cols: partner scale r = c_partner/c_owner
COL_PBI = 35 + 4         # N_PAIR cols: partner bias  -r*xs_partner
COL_G = 43               # N_SLOTS fixup compare values
COL_V = COL_G + N_SLOTS  # N_SLOTS fixup replacement values

_CACHE = {}


def _register_custom_ops():
    if "ops" in _CACHE:
        return _CACHE["ops"]
    import concourse.dve_ops as dve_ops
    from concourse.dve_spec import Spec, Src0, Src1, C0, C1, relu, select, eq, lower
    from concourse.dve_spec import _has_src1
    from concourse.dve_uop import DveOpSpec

    def make(name, spec):
        if name in dve_ops._SUB_OPCODE_FOR_NAME:
            return next(o for o in dve_ops.OPS if o.name == name)
        row = dve_ops._CUSTOM_DVE_ROW_BASE + len(dve_ops.OPS)
        shas = {}
        for ver in ("v3", "v4"):
            s = DveOpSpec(name=name, opcode=row,
                          uops=lower(spec, ver=ver), rd1_en=_has_src1(spec))
            shas[ver] = s.sha(ver)
        op = dve_ops.DveOp(name, spec, subdim=False, uops_sha=shas)
        dve_ops.OPS.append(op)
        dve_ops._SUB_OPCODE_FOR_NAME[name] = row
        dve_ops.CUSTOM_DVE_SPECS[name] = spec
        return op

    relu_mac = make("RELU_MAC_PWA", Spec(
        body=relu(Src0 + C0) * C1 + Src1,
        reference=lambda in0, in1, s0, s1, imm2:
            np.maximum(in0.astype(np.float32) + s0, 0) * s1 + in1,
    ))
    eq_sel = make("EQ_SELECT_PWA", Spec(
        body=select(eq(Src0, C0), C1, Src1),
        reference=lambda in0, in1, s0, s1, imm2:
            np.where(in0 == s0, np.float32(s1), in1).astype(np.float32),
    ))
    _CACHE["ops"] = (relu_mac, eq_sel)
    return _CACHE["ops"]


def _build_program(use_act_seed):
    relu_mac, eq_sel = _register_custom_ops()
    nc = bacc.Bacc(None, target_bir_lowering=False, debug=False)
    xin = nc.declare_dram_parameter("xin", [P, FREE], F32, isOutput=False)
    consts = nc.declare_dram_parameter("consts", [P, NCOL], F32, isOutput=False)
    wmats = nc.declare_dram_parameter("wmats", [P, N_PE * P], F32, isOutput=False)
    xout = nc.declare_dram_parameter("xout", [P, FREE], F32, isOutput=True)

    with tile.TileContext(nc) as tc, ExitStack() as ctx:
        const_pool = ctx.enter_context(tc.tile_pool(name="const", bufs=1))
        x_pool = ctx.enter_context(tc.tile_pool(name="x", bufs=3))
        term_pool = ctx.enter_context(tc.tile_pool(name="term", bufs=6))
        acc_pool = ctx.enter_context(tc.tile_pool(name="acc", bufs=3))
        out_pool = ctx.enter_context(tc.tile_pool(name="out", bufs=3))
        psum_pool = ctx.enter_context(
            tc.tile_pool(name="psum", bufs=2, space=bass.MemorySpace.PSUM)
        )

        cs = const_pool.tile([P, NCOL], F32)
        nc.sync.dma_start(cs[:], consts[:])
        ws = const_pool.tile([P, N_PE * P], F32)
        nc.sync.dma_start(ws[:], wmats[:])

        col = lambda i: cs[:, i : i + 1]
        wmat = lambda i: ws[:, i * P : (i + 1) * P]

        for st in range(FREE // ST):
            sl = slice(st * ST, (st + 1) * ST)
            xt = x_pool.tile([P, ST], F32)
            nc.sync.dma_start(xt[:], xin[:, sl])

            # ACT: unscaled relu produces for the PE owner terms. The last
            # N_PAIR owners also receive a scaled partner term via accum-DMA
            # (r*relu(x-xs_p), r=c_p/c_owner>0 by same-sign pairing) so one
            # c_owner*I matmul group carries two terms.
            terms = []
            for i in range(N_PE):
                t = term_pool.tile([P, ST], F32, tag="term")
                nc.scalar.activation(
                    t[:], xt[:], AF.Relu,
                    bias=col(COL_BIAS + 1 + N_MAC + i), scale=1.0,
                )
                terms.append(t)
            for i in range(N_PAIR):
                s = term_pool.tile([P, ST], F32, tag="pterm")
                nc.scalar.activation(
                    s[:], xt[:], AF.Relu,
                    bias=col(COL_PBI + i), scale=col(COL_PSC + i),
                )
                nc.gpsimd.dma_start(
                    terms[N_PE - N_PAIR + i][:], s[:], accum_op=OP.add
                )
            ps = psum_pool.tile([P, ST], F32)
            for ti, t in enumerate(terms):
                for c in range(ST // CHUNK):
                    cc = slice(c * CHUNK, (c + 1) * CHUNK)
                    nc.tensor.matmul(
                        ps[:, cc], wmat(ti), t[:, cc],
                        start=(ti == 0), stop=(ti == N_PE - 1),
                    )

            # ACT: scaled seed term  c_seed*relu(x - xs_seed)  (c_seed > 0);
            # when A != 0 the seed is instead A*x + c_seed*relu(...) done in
            # two ACT passes (Identity then the chain folds the seed term).
            acc = acc_pool.tile([P, ST], F32)
            if use_act_seed:
                nc.scalar.activation(
                    acc[:], xt[:], AF.Relu,
                    bias=col(COL_SEEDB), scale=col(COL_C + 0),
                )
            else:
                nc.scalar.activation(
                    acc[:], xt[:], AF.Identity, bias=0.0, scale=col(COL_A)
                )
                nc.vector._custom_dve(
                    relu_mac, out=acc[:], in0=xt[:], in1=acc[:],
                    s0=col(COL_BIAS + 0), s1=col(COL_C + 0),
                )

            # DVE fused chain
            for i in range(1, 1 + N_MAC):
                nc.vector._custom_dve(
                    relu_mac, out=acc[:], in0=xt[:], in1=acc[:],
                    s0=col(COL_BIAS + i), s1=col(COL_C + i),
                )

            # merge psum + B into acc -> out, then exact-hit fixups
            ot = out_pool.tile([P, ST], F32)
            nc.vector.affine_then_add(ot[:], ps[:], acc[:], 1.0, col(COL_B))
            for s in range(N_SLOTS):
                nc.vector._custom_dve(
                    eq_sel, out=ot[:], in0=xt[:], in1=ot[:],
                    s0=col(COL_G + s), s1=col(COL_V + s),
                )

            nc.sync.dma_start(xout[:, sl], ot[:])

    nc.compile()
    return nc


def _host_consts(x, xs, slopes, ys):
    xs64 = xs.astype(np.float64)
    ys64 = ys.astype(np.float64)
    s0, s1 = float(slopes[0]), float(slopes[1])
    m = np.diff(ys64) / np.diff(xs64)
    c_raw = np.empty(16, np.float64)
    c_raw[0] = m[0] - s0
    c_raw[1:15] = np.diff(m)
    c_raw[15] = s1 - m[14]

    # term order: seed (a positive-c term) first, then chain, then PE terms.
    # The scaled-ACT seed needs c_seed > 0 and A == 0; otherwise the general
    # path (Identity seed + an extra chain MAC for term 0) is used.
    pos = [k for k in range(16) if c_raw[k] > 0]
    use_act_seed = (s0 == 0.0) and bool(pos)
    seed_k = pos[0] if pos else 0
    rest = [k for k in range(16) if k != seed_k]
    # pick the PE set (last N_PE + N_PAIR of rest) so that it contains
    # N_PAIR same-sign (owner, partner) pairs; among any 8 terms at least
    # 4 share a sign, so 2 pairs always exist.
    pe_set = rest[N_MAC:]
    assert len(pe_set) == N_PE + N_PAIR
    pos_pe = sorted((k for k in pe_set if c_raw[k] > 0), key=lambda k: abs(c_raw[k]))
    neg_pe = sorted((k for k in pe_set if c_raw[k] <= 0), key=lambda k: abs(c_raw[k]))
    pairs, used = [], set()
    for grp in (pos_pe, neg_pe):
        while len(grp) >= 2 and len(pairs) < N_PAIR:
            o, p = grp.pop(), grp.pop()
            pairs.append((o, p)); used.update((o, p))
    assert len(pairs) == N_PAIR, (pos_pe, neg_pe)
    plain = [k for k in pe_set if k not in used]
    owners = plain + [o for o, _ in pairs]          # N_PE owners, paired last
    partners = [p for _, p in pairs]                # N_PAIR partners
    order = [seed_k] + rest[:N_MAC] + owners + partners
    assert len(order) == 1 + N_MAC + N_PE + N_PAIR == 16

    consts = np.zeros(NCOL, np.float64)
    consts[COL_A] = s0
    consts[COL_B] = ys64[0] - s0 * xs64[0]
    for i, k in enumerate(order):
        consts[COL_BIAS + i] = -xs64[k]
        consts[COL_C + i] = c_raw[k]
    consts[COL_SEEDB] = -c_raw[seed_k] * xs64[seed_k]
    for i, (o, p) in enumerate(pairs):
        r = c_raw[p] / c_raw[o]
        assert r > 0
        consts[COL_PSC + i] = r
        consts[COL_PBI + i] = -r * xs64[p]

    # exact-hit fixups: reference's tie-breaking at x == xs[j], j=1..15
    consts[COL_G : COL_G + N_SLOTS] = UNUSED_G
    host_patches = []
    slot = 0
    xs32, ys32 = xs.astype(np.float32), ys.astype(np.float32)
    for j in range(1, 16):
        g = xs32[j]
        if not np.any(x == g):
            continue
        l, r = (j - 1, j + 1) if j < 15 else (14, 0)
        denom = np.float32(xs32[r] - xs32[l])
        v = np.float32(
            ys32[l] + (g - xs32[l]) * np.float32(ys32[r] - ys32[l]) / denom
        )
        if slot < N_SLOTS:
            consts[COL_G + slot] = np.float64(g)
            consts[COL_V + slot] = np.float64(v)
            slot += 1
        else:
            host_patches.append((g, v))

    wm = np.zeros((N_PE, P, P), np.float64)
    for i in range(N_PE):
        wm[i] = c_raw[owners[i]] * np.eye(P)

    consts_np = np.broadcast_to(consts.astype(np.float32), (P, NCOL)).copy()
    wmats_np = np.ascontiguousarray(
        wm.astype(np.float32).transpose(1, 0, 2).reshape(P, N_PE * P)
    )
    return consts_np, wmats_np, host_patches, use_act_seed


def kernel(x, xs, slopes, ys):
    x = np.ascontiguousarray(x, np.float32)
    assert x.size == N_CORES * P * FREE, f"unexpected x shape {x.shape}"
    consts_np, wmats_np, host_patches, use_act_seed = _host_consts(x, xs, slopes, ys)
    if _CACHE.get("use_act_seed") != use_act_seed:
        _CACHE["use_act_seed"] = use_act_seed
        _CACHE["nc"] = _build_program(use_act_seed)
    nc = _CACHE["nc"]

    shards = x.reshape(N_CORES, P, FREE)
    in_maps = [
        {"xin": shards[i], "consts": consts_np, "wmats": wmats_np}
        for i in range(N_CORES)
    ]
    import os
    res = run_bass_kernel_spmd(
        nc, in_maps, list(range(N_CORES)),
        trace=bool(int(os.environ.get("KERNEL_TRACE", "0"))),
    )
    _CACHE["last_results"] = res
    out = np.stack([res.results[i]["xout"] for i in range(N_CORES)])
    out = out.reshape(x.shape)
    for g, v in host_patches:  # only if >N_SLOTS distinct exact-hit values
        out[x == g] = v
    return out


# revision 32
# speedup vs baseline: 1.3026x; 1.3026x over previous
"""Piecewise-linear activation (uniform 16-point grid) on 8 trn2 NeuronCores.

Math: the reference is (except at exact grid hits) the continuous PWL function
    f(x) = A*x + B + sum_k c_k * relu(x - xs_k),   k = 0..15
with
    m_j   = (ys[j+1]-ys[j])/(xs[j+1]-xs[j])      (15 interior slopes)
    c_0   = m_0 - slopes[0]
    c_k   = m_k - m_{k-1}                         (k=1..14)
    c_15  = slopes[1] - m_14
    A     = slopes[0],  B = ys[0] - slopes[0]*xs[0]
At an exact interior grid hit x == xs[j] (j=1..15) the reference's
argmin/argmax tie-breaking yields the two-segment-wide interpolation value
(discontinuous from f); those are patched with EQ_SELECT fixup slots.

Engine split per 128x2048 supertile (driven by measured per-instr costs; the
PE array is HAM power-throttled and fp32 matmul is 2-pass, ~4.5us/st/term;
DVE RELU_MAC fuses produce+scale+accumulate into one ~2.4us 1x pass):
  ACT : one seed term c_k*relu(x-xs_k) (positive c_k, scale rides the ACT)
        + unscaled relu produces for the PE terms
        (+ A*x seed via Identity when A != 0)
  DVE : RELU_MAC chain for 9 terms on top of the seed, PSUM merge via
        affine_then_add whose bias slot carries B, then EQ_SELECT fixups
  PE  : 6 terms accumulate into PSUM via c_k*I fp32 matmuls
  (GPSIMD tensor ops are rejected by walrus codegen / crash the NC;
   accum-DMA works but serializes the output tile - both unused)
"""

import numpy as np
from contextlib import ExitStack

import concourse.bass as bass
import concourse.bacc as bacc
import concourse.tile as tile
from concourse import mybir
from concourse.bass_utils import run_bass_kernel_spmd

F32 = mybir.dt.float32
AF = mybir.ActivationFunctionType
OP = mybir.AluOpType

N_CORES = 8
P = 128
FREE = 8192              # elements per partition per core (1024*1024/128)
ST = 2048                # supertile free size
CHUNK = 512              # one PSUM bank worth of fp32

N_MAC = 7                # terms fused on the DVE RELU_MAC chain
N_PE = 6                 # PE matmul groups (owner terms)
N_PAIR = 2               # partner terms DMA-accumulated into the last owners
N_SLOTS = 2              # exact-hit fixup slots
UNUSED_G = 1.0e30        # sentinel no input value ever equals

# term order: [seed (positive c)] + N_MAC chain terms + N_PE terms
# consts column layout (each column replicated over 128 partitions):
NCOL = 48
COL_A = 0
COL_B = 1
COL_BIAS = 2             # 16 cols: -xs_k in term order
COL_C = 18               # 16 cols: c_k in term order
COL_SEEDB = 34           # -c_seed * xs_seed (ACT bias for the scaled seed)
COL_PSC = 35             # N_PAIR cols: partner scale r = c_partner/c_owner
COL_PBI = 35 + 4         # N_PAIR cols: partner bias  -r*xs_partner
COL_G = 43               # N_SLOTS fixup compare values
COL_V = COL_G + N_SLOTS  # N_SLOTS fixup replacement values

_CACHE = {}


def _register_custom_ops():
    if "ops" in _CACHE:
        return _CACHE["ops"]
    import concourse.dve_ops as dve_ops
    from concourse.dve_spec import Spec, Src0, Src1, C0, C1, relu, select, eq, lower
    from concourse.dve_spec import _has_src1
    from concourse.dve_uop import DveOpSpec

    def make(name, spec):
        if name in dve_ops._SUB_OPCODE_FOR_NAME:
            return next(o for o in dve_ops.OPS if o.name == name)
        row = dve_ops._CUSTOM_DVE_ROW_BASE + len(dve_ops.OPS)
        shas = {}
        for ver in ("v3", "v4"):
            s = DveOpSpec(name=name, opcode=row,
                          uops=lower(spec, ver=ver), rd1_en=_has_src1(spec))
            shas[ver] = s.sha(ver)
        op = dve_ops.DveOp(name, spec, subdim=False, uops_sha=shas)
        dve_ops.OPS.append(op)
        dve_ops._SUB_OPCODE_FOR_NAME[name] = row
        dve_ops.CUSTOM_DVE_SPECS[name] = spec
        return op

    relu_mac = make("RELU_MAC_PWA", Spec(
        body=relu(Src0 + C0) * C1 + Src1,
        reference=lambda in0, in1, s0, s1, imm2:
            np.maximum(in0.astype(np.float32) + s0, 0) * s1 + in1,
    ))
    eq_sel = make("EQ_SELECT_PWA", Spec(
        body=select(eq(Src0, C0), C1, Src1),
        reference=lambda in0, in1, s0, s1, imm2:
            np.where(in0 == s0, np.float32(s1), in1).astype(np.float32),
    ))
    _CACHE["ops"] = (relu_mac, eq_sel)
    return _CACHE["ops"]


def _build_program(use_act_seed):
    relu_mac, eq_sel = _register_custom_ops()
    nc = bacc.Bacc(None, target_bir_lowering=False, debug=False)
    xin = nc.declare_dram_parameter("xin", [P, FREE], F32, isOutput=False)
    consts = nc.declare_dram_parameter("consts", [P, NCOL], F32, isOutput=False)
    wmats = nc.declare_dram_parameter("wmats", [P, N_PE * P], F32, isOutput=False)
    xout = nc.declare_dram_parameter("xout", [P, FREE], F32, isOutput=True)

    with tile.TileContext(nc) as tc, ExitStack() as ctx:
        const_pool = ctx.enter_context(tc.tile_pool(name="const", bufs=1))
        x_pool = ctx.enter_context(tc.tile_pool(name="x", bufs=3))
        term_pool = ctx.enter_context(tc.tile_pool(name="term", bufs=6))
        acc_pool = ctx.enter_context(tc.tile_pool(name="acc", bufs=3))
        out_pool = ctx.enter_context(tc.tile_pool(name="out", bufs=3))
        psum_pool = ctx.enter_context(
            tc.tile_pool(name="psum", bufs=2, space=bass.MemorySpace.PSUM)
        )

        cs = const_pool.tile([P, NCOL], F32)
        nc.sync.dma_start(cs[:], consts[:])
        ws = const_pool.tile([P, N_PE * P], F32)
        nc.sync.dma_start(ws[:], wmats[:])

        col = lambda i: cs[:, i : i + 1]
        wmat = lambda i: ws[:, i * P : (i + 1) * P]

        for st in range(FREE // ST):
            sl = slice(st * ST, (st + 1) * ST)
            xt = x_pool.tile([P, ST], F32)
            nc.sync.dma_start(xt[:], xin[:, sl])

            # ACT: scaled seed term  c_seed*relu(x - xs_seed)  (c_seed > 0);
            # when A != 0 the seed is instead A*x + c_seed*relu(...) done in
            # two ACT passes (Identity then the chain folds the seed term).
            acc = acc_pool.tile([P, ST], F32)
            if use_act_seed:
                nc.scalar.activation(
                    acc[:], xt[:], AF.Relu,
                    bias=col(COL_SEEDB), scale=col(COL_C + 0),
                )
            else:
                nc.scalar.activation(
                    acc[:], xt[:], AF.Identity, bias=0.0, scale=col(COL_A)
                )
                nc.vector._custom_dve(
                    relu_mac, out=acc[:], in0=xt[:], in1=acc[:],
                    s0=col(COL_BIAS + 0), s1=col(COL_C + 0),
                )

            # DVE fused chain
            for i in range(1, 1 + N_MAC):
                nc.vector._custom_dve(
                    relu_mac, out=acc[:], in0=xt[:], in1=acc[:],
                    s0=col(COL_BIAS + i), s1=col(COL_C + i),
                )

            # ACT: unscaled relu produces for the PE owner terms. The last
            # N_PAIR owners also receive a scaled partner term via accum-DMA
            # (r*relu(x-xs_p), r=c_p/c_owner>0 by same-sign pairing) so one
            # c_owner*I matmul group carries two terms.
            terms = []
            for i in range(N_PE):
                t = term_pool.tile([P, ST], F32, tag="term")
                nc.scalar.activation(
                    t[:], xt[:], AF.Relu,
                    bias=col(COL_BIAS + 1 + N_MAC + i), scale=1.0,
                )
                terms.append(t)
            for i in range(N_PAIR):
                s = term_pool.tile([P, ST], F32, tag="pterm")
                nc.scalar.activation(
                    s[:], xt[:], AF.Relu,
                    bias=col(COL_PBI + i), scale=col(COL_PSC + i),
                )
                nc.gpsimd.dma_start(
                    terms[N_PE - N_PAIR + i][:], s[:], accum_op=OP.add
                )
            ps = psum_pool.tile([P, ST], F32)
            for ti, t in enumerate(terms):
                for c in range(ST // CHUNK):
                    cc = slice(c * CHUNK, (c + 1) * CHUNK)
                    nc.tensor.matmul(
                        ps[:, cc], wmat(ti), t[:, cc],
                        start=(ti == 0), stop=(ti == N_PE - 1),
                    )

            # merge psum + B into acc -> out, then exact-hit fixups
            ot = out_pool.tile([P, ST], F32)
            nc.vector.affine_then_add(ot[:], ps[:], acc[:], 1.0, col(COL_B))
            for s in range(N_SLOTS):
                nc.vector._custom_dve(
                    eq_sel, out=ot[:], in0=xt[:], in1=ot[:],
                    s0=col(COL_G + s), s1=col(COL_V + s),
                )

            nc.sync.dma_start(xout[:, sl], ot[:])

    nc.compile()
    return nc


def _host_consts(x, xs, slopes, ys):
    xs64 = xs.astype(np.float64)
    ys64 = ys.astype(np.float64)
    s0, s1 = float(slopes[0]), float(slopes[1])
    m = np.diff(ys64) / np.diff(xs64)
    c_raw = np.empty(16, np.float64)
    c_raw[0] = m[0] - s0
    c_raw[1:15] = np.diff(m)
    c_raw[15] = s1 - m[14]

    # term order: seed (a positive-c term) first, then chain, then PE terms.
    # The scaled-ACT seed needs c_seed > 0 and A == 0; otherwise the general
    # path (Identity seed + an extra chain MAC for term 0) is used.
    pos = [k for k in range(16) if c_raw[k] > 0]
    use_act_seed = (s0 == 0.0) and bool(pos)
    seed_k = pos[0] if pos else 0
    rest = [k for k in range(16) if k != seed_k]
    # pick the PE set (last N_PE + N_PAIR of rest) so that it contains
    # N_PAIR same-sign (owner, partner) pairs; among any 8 terms at least
    # 4 share a sign, so 2 pairs always exist.
    pe_set = rest[N_MAC:]
    assert len(pe_set) == N_PE + N_PAIR
    pos_pe = sorted((k for k in pe_set if c_raw[k] > 0), key=lambda k: abs(c_raw[k]))
    neg_pe = sorted((k for k in pe_set if c_raw[k] <= 0), key=lambda k: abs(c_raw[k]))
    pairs, used = [], set()
    for grp in (pos_pe, neg_pe):
        while len(grp) >= 2 and len(pairs) < N_PAIR:
            o, p = grp.pop(), grp.pop()
            pairs.append((o, p)); used.update((o, p))
    assert len(pairs) == N_PAIR, (pos_pe, neg_pe)
    plain = [k for k in pe_set if k not in used]
    owners = plain + [o for o, _ in pairs]          # N_PE owners, paired last
    partners = [p for _, p in pairs]                # N_PAIR partners
    order = [seed_k] + rest[:N_MAC] + owners + partners
    assert len(order) == 1 + N_MAC + N_PE + N_PAIR == 16

    consts = np.zeros(NCOL, np.float64)
    consts[COL_A] = s0
    consts[COL_B] = ys64[0] - s0 * xs64[0]
    for i, k in enumerate(order):
        consts[COL_BIAS + i] = -xs64[k]
        consts[COL_C + i] = c_raw[k]
    consts[COL_SEEDB] = -c_raw[seed_k] * xs64[seed_k]
    for i, (o, p) in enumerate(pairs):
        r = c_raw[p] / c_raw[o]
        assert r > 0
        consts[COL_PSC + i] = r
        consts[COL_PBI + i] = -r * xs64[p]

    # exact-hit fixups: reference's tie-breaking at x == xs[j], j=1..15
    consts[COL_G : COL_G + N_SLOTS] = UNUSED_G
    host_patches = []
    slot = 0
    xs32, ys32 = xs.astype(np.float32), ys.astype(np.float32)
    for j in range(1, 16):
        g = xs32[j]
        if not np.any(x == g):
            continue
        l, r = (j - 1, j + 1) if j < 15 else (14, 0)
        denom = np.float32(xs32[r] - xs32[l])
        v = np.float32(
            ys32[l] + (g - xs32[l]) * np.float32(ys32[r] - ys32[l]) / denom
        )
        if slot < N_SLOTS:
            consts[COL_G + slot] = np.float64(g)
            consts[COL_V + slot] = np.float64(v)
            slot += 1
        else:
            host_patches.append((g, v))

    wm = np.zeros((N_PE, P, P), np.float64)
    for i in range(N_PE):
        wm[i] = c_raw[owners[i]] * np.eye(P)

    consts_np = np.broadcast_to(consts.astype(np.float32), (P, NCOL)).copy()
    wmats_np = np.ascontiguousarray(
        wm.astype(np.float32).transpose(1, 0, 2).reshape(P, N_PE * P)
    )
    return consts_np, wmats_np, host_patches, use_act_seed


def kernel(x, xs, slopes, ys):
    x = np.ascontiguousarray(x, np.float32)
    assert x.size == N_CORES * P * FREE, f"unexpected x shape {x.shape}"
    consts_np, wmats_np, host_patches, use_act_seed = _host_consts(x, xs, slopes, ys)
    if _CACHE.get("use_act_seed") != use_act_seed:
        _CACHE["use_act_seed"] = use_act_seed
        _CACHE["nc"] = _build_program(use_act_seed)
    nc = _CACHE["nc"]

    shards = x.reshape(N_CORES, P, FREE)
    in_maps = [
        {"xin": shards[i], "consts": consts_np, "wmats": wmats_np}
        for i in range(N_CORES)
    ]
    import os
    res = run_bass_kernel_spmd(
        nc, in_maps, list(range(N_CORES)),
        trace=bool(int(os.environ.get("KERNEL_TRACE", "0"))),
    )
    _CACHE["last_results"] = res
    out = np.stack([res.results[i]["xout"] for i in range(N_CORES)])
    out = out.reshape(x.shape)
    for g, v in host_patches:  # only if >N_SLOTS distinct exact-hit values
        out[x == g] = v
    return out
